# revision 20
# baseline (speedup 1.0000x reference)
"""Trainium2 Bass kernel for knn_interpolate (k=3) + 2-layer MLP (FPModule).

Contract: kernel(**inputs) takes the FULL unsharded inputs (as produced by
setup_inputs()) and returns the FULL output, matching the reference:
    (h [16384,512] f32, pos_skip [16384,3] f32, batch_skip [16384] i64)

Sharding: 16384 queries split across 8 cores (2048 each). Batches are
contiguous & equal-sized (4 batches x 4096 queries / 1024 coarse points), so
core c serves queries [c*2048,(c+1)*2048) which all live in batch c//2; the
core only receives that batch's coarse points (masked KNN == per-batch KNN).

Per-core device algorithm:
  1. Dneg[q,c] = -||q-c||^2 via one K=5 matmul with augmented operands
     lhsT rows [qx,qy,qz,qq,1], rhs rows [2cx,2cy,2cz,-1,-cc].
  2. max8 over each [128,1024] row block -> top-8 candidates by PE distance;
     max_index -> their column indices (ties -> lowest index, like top_k).
  3. Take top-4 candidates, gather their [features | position] rows with ONE
     batched dma_gather, recompute exact fp32 d2 on-chip, re-rank exactly,
     compute 1/d2 weights from the exact values (masked to the exact top-3).
  4. xi = sum_j wn_j * xk_j (fused scalar_tensor_tensor chain), then
     h = relu(relu([xi,x_skip]@W1+b1)@W2+b2) with activations kept
     feature-major (PE transposes) so matmuls contract on partitions.
"""

import os
from contextlib import ExitStack

import numpy as np

NQ = 2048          # queries per core
NCC = 1024         # coarse points per core (one batch)
CF = 256           # coarse feature dim
SF = 128           # skip feature dim
HID = 512          # MLP hidden dim
KNN = 3
NCAND = 4          # candidates gathered per query (exact re-rank pool)
ELEM = 320         # padded row of the gather table: [x_c(256) | pos(3) | pad]
QT = NQ // 128     # 16 query tiles
CT = NCC // 128    # 8 coarse chunks
N_CORES = 8
NSLOT = QT * NCAND          # 64 gather slots per partition
NIDX = NSLOT * 128          # 8192 gathered rows per core

_CACHE = {}


def _build_nc():
    repeat = int(os.environ.get("KERNEL_REPEAT", "1"))

    import concourse.bass as bass
    import concourse.bacc as bacc
    import concourse.mybir as mybir
    import concourse.tile as tile

    f32 = mybir.dt.float32
    u32 = mybir.dt.uint32
    i16 = mybir.dt.int16
    AF = mybir.ActivationFunctionType
    OP = mybir.AluOpType
    X = mybir.AxisListType.X

    nc = bacc.Bacc("TRN2", target_bir_lowering=False, debug=False)

    posq = nc.dram_tensor("posq", [NQ, 3], f32, kind="ExternalInput").ap()
    xq = nc.dram_tensor("xq", [NQ, SF], f32, kind="ExternalInput").ap()
    posc = nc.dram_tensor("posc", [NCC, 3], f32, kind="ExternalInput").ap()
    xca = nc.dram_tensor("xca", [NCC, ELEM], f32, kind="ExternalInput").ap()
    w1 = nc.dram_tensor("w1", [CF + SF, HID], f32, kind="ExternalInput").ap()
    b1 = nc.dram_tensor("b1", [HID], f32, kind="ExternalInput").ap()
    w2 = nc.dram_tensor("w2", [HID, HID], f32, kind="ExternalInput").ap()
    b2 = nc.dram_tensor("b2", [HID], f32, kind="ExternalInput").ap()
    out = nc.dram_tensor("out", [NQ, HID], f32, kind="ExternalOutput").ap()
    stage = nc.dram_tensor("stage", [16 * 8 * NSLOT], i16).ap()  # idx bounce [c,g,s]

    from concourse.masks import make_identity

    with tile.TileContext(nc) as tc, ExitStack() as ctx:
        const = ctx.enter_context(tc.tile_pool(name="const", bufs=1))
        sb = ctx.enter_context(tc.tile_pool(name="sb", bufs=3))
        dsb_p = ctx.enter_context(tc.tile_pool(name="dsb", bufs=3))
        xi_p = ctx.enter_context(tc.tile_pool(name="xi", bufs=3))
        out_p = ctx.enter_context(tc.tile_pool(name="outp", bufs=3))
        psD = ctx.enter_context(tc.tile_pool(name="psD", bufs=2, space="PSUM"))
        psT = ctx.enter_context(tc.tile_pool(name="psT", bufs=2, space="PSUM"))
        psM = ctx.enter_context(tc.tile_pool(name="psM", bufs=2, space="PSUM"))

        # ---------- constants / persistent state ----------
        ident = const.tile([128, 128], f32)
        make_identity(nc, ident[:])

        w1_sb = const.tile([128, 3, HID], f32)
        nc.sync.dma_start(w1_sb[:], w1.rearrange("(kc p) h -> p kc h", p=128))
        w2_sb = const.tile([128, 4, HID], f32)
        nc.sync.dma_start(w2_sb[:], w2.rearrange("(kc p) h -> p kc h", p=128))
        b1_sb = const.tile([128, 4], f32)
        nc.sync.dma_start(b1_sb[:], b1.rearrange("(t p) -> p t", p=128))
        b2_sb = const.tile([1, HID], f32)
        nc.sync.dma_start(b2_sb[:], b2.rearrange("(o h) -> o h", o=1))
        ones1 = const.tile([1, 128], f32)
        nc.vector.memset(ones1[:], 1.0)

        posq_sb = const.tile([128, QT, 3], f32)
        nc.sync.dma_start(posq_sb[:], posq.rearrange("(t p) c -> p t c", p=128))
        posc_sb = const.tile([128, CT, 3], f32)
        nc.sync.dma_start(posc_sb[:], posc.rearrange("(t p) c -> p t c", p=128))

        # query positions replicated per candidate slot: [128, QT, NCAND, 3]
        posq_rep = const.tile([128, QT, NCAND, 3], f32)
        for j in range(NCAND):
            nc.vector.tensor_copy(posq_rep[:, :, j, :], posq_sb[:])

        # ---------- query-side augmented operand ----------
        # aug[p, t, :] = [qx, qy, qz, qq, 1, 0, 0, 0] for query t*128+p
        aug = const.tile([128, QT, 8], f32)
        nc.vector.tensor_copy(aug[:, :, 0:3], posq_sb[:])
        sqq = sb.tile([128, QT, 3], f32, tag="sqq")
        nc.vector.tensor_mul(sqq[:], posq_sb[:], posq_sb[:])
        nc.vector.reduce_sum(aug[:, :, 3:4], sqq[:], axis=X)
        nc.vector.memset(aug[:, :, 4:5], 1.0)
        nc.vector.memset(aug[:, :, 5:8], 0.0)

        augT = const.tile([8, QT * 128], f32)
        for t in range(QT):
            tp = psT.tile([8, 128], f32, tag="tps")
            nc.tensor.transpose(tp[:], aug[:, t, :], ident[:])
            nc.scalar.activation(augT[:, t * 128:(t + 1) * 128], tp[:], AF.Copy)

        # ---------- coarse-side rhs operand [8, NCC] ----------
        # rows: [2cx, 2cy, 2cz, -1, -cc, 0, 0, 0]
        cst = const.tile([128, CT, 8], f32)
        nc.scalar.mul(cst[:, :, 0:3], posc_sb[:], 2.0)
        nc.vector.memset(cst[:, :, 3:4], -1.0)
        sqc = sb.tile([128, CT, 3], f32, tag="sqc")
        nc.vector.tensor_mul(sqc[:], posc_sb[:], posc_sb[:])
        nc.vector.reduce_sum(cst[:, :, 4:5], sqc[:], axis=X, negate=True)
        nc.vector.memset(cst[:, :, 5:8], 0.0)
        rhsD = const.tile([8, NCC], f32)
        for u in range(CT):
            tp = psT.tile([8, 128], f32, tag="tps")
            nc.tensor.transpose(tp[:], cst[:, u, :], ident[:])
            nc.scalar.activation(rhsD[:, u * 128:(u + 1) * 128], tp[:], AF.Copy)

        for _rep in range(repeat):
            # ---------- distances + top-8 candidates ----------
            mx = const.tile([128, QT * 8], f32)
            ix = const.tile([128, QT * 8], u32)
            for t in range(QT):
                psd = psD.tile([128, NCC], f32, tag="psd")
                lhs = augT[:, t * 128:(t + 1) * 128]
                nc.tensor.matmul(psd[:, 0:512], lhs, rhsD[:, 0:512], start=True, stop=True)
                nc.tensor.matmul(psd[:, 512:1024], lhs, rhsD[:, 512:1024], start=True, stop=True)
                dsb = dsb_p.tile([128, NCC], f32, tag="dsb")
                nc.scalar.activation(dsb[:], psd[:], AF.Copy)
                nc.vector.max(mx[:, t * 8:(t + 1) * 8], dsb[:])
                nc.vector.max_index(ix[:, t * 8:(t + 1) * 8], mx[:, t * 8:(t + 1) * 8], dsb[:])

            # ---------- build wrapped int16 index table for dma_gather ----------
            # slot layout: slot = t*NCAND + j holds candidate j of tile t.
            ixc = const.tile([128, NSLOT], i16)
            ix_v = ix[:].rearrange("p (t e) -> p t e", e=8)
            nc.vector.tensor_copy(ixc[:].rearrange("p (t j) -> p t j", j=NCAND),
                                  ix_v[:, :, 0:NCAND])
            # hop1: straight row-major store; stage[p * NSLOT + s] = ixc[p, s]
            nc.sync.dma_start(
                stage.rearrange("(p s) -> p s", s=NSLOT), ixc[:]
            )
            # hop2: per 16-partition group, load the cross-partition regroup
            # raw[c, g*NSLOT + s] = stage[(16g + c) * NSLOT + s]
            raw = const.tile([128, 8 * NSLOT], i16)
            for grp in range(8):
                src_ap = bass.AP(
                    tensor=stage.tensor,
                    offset=0,
                    ap=[[NSLOT, 16], [16 * NSLOT, 8], [1, NSLOT]],
                )
                nc.sync.dma_start(raw[16 * grp:16 * (grp + 1), :], src_ap)
            # hop3: in-SBUF permute (g, s) -> wrapped order (s, g) on DVE
            table = const.tile([128, NIDX // 16], i16)
            nc.vector.tensor_copy(
                table[:].rearrange("p (s g) -> p g s", g=8),
                raw[:].rearrange("p (g s) -> p g s", g=8),
            )

            # ---------- batched gather of [features | position] ----------
            # split into 1024-row chunks (large single gathers wedge the
            # SWDGE descriptor ring)
            xk = const.tile([128, NSLOT, ELEM], f32)
            for kk in range(NIDX // 1024):
                nc.gpsimd.dma_gather(
                    out_ap=xk[:, 8 * kk:8 * (kk + 1), :],
                    in_ap=xca[:, :],
                    idxs_ap=table[:, 64 * kk:64 * (kk + 1)],
                    num_idxs=1024,
                    num_idxs_reg=1024,
                    elem_size=ELEM,
                )

            # ---------- exact re-rank + interpolation weights ----------
            diff = const.tile([128, NSLOT, 3], f32)
            nc.vector.tensor_sub(
                diff[:], xk[:, :, CF:CF + 3],
                posq_rep[:].rearrange("p t j c -> p (t j) c"),
            )
            sqd = const.tile([128, NSLOT, 3], f32)
            nc.vector.tensor_mul(sqd[:], diff[:], diff[:])
            d2e = const.tile([128, QT, NCAND], f32)
            nc.vector.reduce_sum(d2e[:], sqd[:], axis=X)

            # negated + padded to 8 for per-tile max8 exact sort
            dn = const.tile([128, QT, 8], f32)
            nc.vector.memset(dn[:, :, NCAND:8], -1e30)
            nc.vector.tensor_scalar(dn[:, :, 0:NCAND], d2e[:], -1.0, None, op0=OP.mult)
            mxe = const.tile([128, QT * 8], f32)
            for t in range(QT):
                nc.vector.max(mxe[:, t * 8:(t + 1) * 8], dn[:, t, :])
            mxe_v = mxe[:].rearrange("p (t e) -> p t e", e=8)

            # exact third-smallest per query (selection threshold)
            v3e = const.tile([128, QT], f32)
            nc.vector.tensor_scalar(v3e[:], mxe_v[:, :, 2], -1.0, None, op0=OP.mult)
            # masked inverse-distance weights over the NCAND candidates
            d2c = const.tile([128, QT, NCAND], f32)
            nc.vector.tensor_scalar(d2c[:], d2e[:], 1e-16, None, op0=OP.max)
            wraw = const.tile([128, QT, NCAND], f32)
            nc.vector.reciprocal(wraw[:], d2c[:])
            mask = const.tile([128, QT, NCAND], f32)
            for j in range(NCAND):
                nc.vector.tensor_tensor(
                    mask[:, :, j], d2e[:, :, j], v3e[:], op=OP.is_le
                )
            wk = const.tile([128, QT, NCAND], f32)
            nc.vector.tensor_mul(wk[:], wraw[:], mask[:])
            wsum = const.tile([128, QT], f32)
            nc.vector.reduce_sum(wsum[:], wk[:], axis=X)
            wni = const.tile([128, QT], f32)
            nc.vector.reciprocal(wni[:], wsum[:])
            wn = const.tile([128, QT, NCAND], f32)
            for j in range(NCAND):
                nc.vector.tensor_mul(wn[:, :, j], wk[:, :, j], wni[:])

            # ---------- weighted sum + transpose to feature-major ----------
            xiT = const.tile([128, 2, NQ], f32)
            for t in range(QT):
                s0 = t * NCAND
                xi = xi_p.tile([128, CF], f32, tag="xi")
                nc.vector.tensor_scalar_mul(xi[:], xk[:, s0, 0:CF], wn[:, t, 0:1])
                for j in range(1, NCAND):
                    nc.vector.scalar_tensor_tensor(
                        xi[:], xk[:, s0 + j, 0:CF], wn[:, t, j:j + 1], xi[:],
                        op0=OP.mult, op1=OP.add,
                    )
                for fc in range(2):
                    tp = psT.tile([128, 128], f32, tag="tps")
                    nc.tensor.transpose(tp[:], xi[:, fc * 128:(fc + 1) * 128], ident[:])
                    nc.scalar.activation(xiT[:, fc, t * 128:(t + 1) * 128], tp[:], AF.Copy)

            # x_skip transposed to feature-major
            xsT = const.tile([128, NQ], f32)
            for t in range(QT):
                xst = sb.tile([128, SF], f32, tag="xst")
                nc.sync.dma_start(xst[:], xq[t * 128:(t + 1) * 128, :])
                tp = psT.tile([128, 128], f32, tag="tps")
                nc.tensor.transpose(tp[:], xst[:], ident[:])
                nc.scalar.activation(xsT[:, t * 128:(t + 1) * 128], tp[:], AF.Copy)

            # ---------- MLP layer 1: h1T[p, ht, q] ----------
            h1T = const.tile([128, 4, NQ], f32)
            for qc in range(NQ // 512):
                qs = slice(qc * 512, (qc + 1) * 512)
                for ht in range(4):
                    ps = psM.tile([128, 512], f32, tag="psm")
                    for kc in range(3):
                        rhs = xiT[:, kc, qs] if kc < 2 else xsT[:, qs]
                        nc.tensor.matmul(
                            ps[:],
                            w1_sb[:, kc, ht * 128:(ht + 1) * 128],
                            rhs,
                            start=(kc == 0),
                            stop=(kc == 2),
                        )
                    nc.scalar.activation(
                        h1T[:, ht, qs], ps[:], AF.Relu, bias=b1_sb[:, ht:ht + 1]
                    )

            # ---------- MLP layer 2 (query-major out) ----------
            for t in range(QT):
                ps2 = psM.tile([128, 512], f32, tag="psm")
                for kc in range(4):
                    nc.tensor.matmul(
                        ps2[:],
                        h1T[:, kc, t * 128:(t + 1) * 128],
                        w2_sb[:, kc, :],
                        start=(kc == 0),
                        stop=False,
                    )
                nc.tensor.matmul(ps2[:], ones1[:], b2_sb[:], start=False, stop=True)
                ot = out_p.tile([128, HID], f32, tag="ot")
                nc.scalar.activation(ot[:], ps2[:], AF.Relu)
                nc.sync.dma_start(out[t * 128:(t + 1) * 128, :], ot[:])

    nc.finalize()
    return nc


def get_nc():
    if "nc" not in _CACHE:
        _CACHE["nc"] = _build_nc()
    return _CACHE["nc"]


def make_in_maps(x, pos, x_skip, pos_skip, W1, b1, W2, b2):
    c32 = lambda a: np.ascontiguousarray(np.asarray(a), dtype=np.float32)
    x = c32(x)
    pos = c32(pos)
    xca_full = np.zeros((x.shape[0], ELEM), np.float32)
    xca_full[:, :CF] = x
    xca_full[:, CF:CF + 3] = pos
    maps = []
    for c in range(N_CORES):
        b = (c * NQ) // (len(pos_skip) // 4)  # batch of this core's query slice
        maps.append(
            {
                "posq": c32(pos_skip[c * NQ:(c + 1) * NQ]),
                "xq": c32(x_skip[c * NQ:(c + 1) * NQ]),
                "posc": c32(pos[b * NCC:(b + 1) * NCC]),
                "xca": np.ascontiguousarray(xca_full[b * NCC:(b + 1) * NCC]),
                "w1": c32(W1),
                "b1": c32(b1),
                "w2": c32(W2),
                "b2": c32(b2),
            }
        )
    return maps


def _expected_layout(batch, batch_skip, k):
    """True iff inputs match the layout this kernel was compiled for."""
    batch = np.asarray(batch)
    batch_skip = np.asarray(batch_skip)
    if int(k) != KNN:
        return False
    if batch.shape != (4096,) or batch_skip.shape != (16384,):
        return False
    exp_b = np.repeat(np.arange(4), 1024)
    exp_bs = np.repeat(np.arange(4), 4096)
    return bool(np.array_equal(batch, exp_b) and np.array_equal(batch_skip, exp_bs))


def _numpy_fallback(x, pos, batch, x_skip, pos_skip, batch_skip, W1, b1, W2, b2, k):
    x, pos, x_skip, pos_skip = (np.asarray(a, np.float64) for a in (x, pos, x_skip, pos_skip))
    batch = np.asarray(batch)
    batch_skip = np.asarray(batch_skip)
    d2 = (
        (pos_skip**2).sum(-1)[:, None]
        + (pos**2).sum(-1)[None, :]
        - 2.0 * (pos_skip @ pos.T)
    )
    d2 = np.maximum(d2, 0.0)
    d2 = d2 + np.where(batch_skip[:, None] != batch[None, :], 1e10, 0.0)
    idx = np.argsort(d2, axis=1, kind="stable")[:, : int(k)]
    d2k = np.take_along_axis(d2, idx, axis=1)
    w = 1.0 / np.clip(d2k, 1e-16, None)
    xk = x[idx]
    xi = (w[..., None] * xk).sum(1) / w.sum(1, keepdims=True)
    h = np.concatenate([xi, np.asarray(x_skip, np.float64)], axis=1)
    h = np.maximum(h @ np.asarray(W1, np.float64) + np.asarray(b1, np.float64), 0.0)
    h = np.maximum(h @ np.asarray(W2, np.float64) + np.asarray(b2, np.float64), 0.0)
    return (
        h.astype(np.float32),
        np.asarray(pos_skip, np.float32),
        batch_skip,
    )


def kernel(x, pos, batch, x_skip, pos_skip, batch_skip, W1, b1, W2, b2, k, **run_kw):
    x = np.asarray(x)
    pos = np.asarray(pos)
    x_skip = np.asarray(x_skip)
    pos_skip = np.asarray(pos_skip)
    if not _expected_layout(batch, batch_skip, k):
        return _numpy_fallback(
            x, pos, batch, x_skip, pos_skip, batch_skip, W1, b1, W2, b2, k
        )

    in_maps = make_in_maps(x, pos, x_skip, pos_skip, W1, b1, W2, b2)
    try:
        if "runner" not in _CACHE:
            _CACHE["runner"] = make_runner(get_nc(), n_cores=N_CORES)
        results = _CACHE["runner"](in_maps)
    except Exception:
        _CACHE.pop("runner", None)
        from concourse.bass_utils import run_bass_kernel_spmd

        results = run_bass_kernel_spmd(get_nc(), in_maps, list(range(N_CORES))).results
    h = np.concatenate([results[c]["out"] for c in range(N_CORES)], axis=0)
    return h, np.asarray(pos_skip), np.asarray(batch_skip)


def make_runner(nc, n_cores=8):
    """Build a reusable jitted SPMD executor for `nc` on the first n_cores
    devices. Returns run(in_maps) -> list[dict[name, np.ndarray]]."""
    import jax
    from jax.sharding import Mesh, PartitionSpec, NamedSharding

    try:
        from jax.experimental.shard_map import shard_map
    except ImportError:
        from jax import shard_map
    import concourse.mybir as mybir
    from concourse.bass2jax import (
        _bass_exec_p,
        install_neuronx_cc_hook,
        partition_id_tensor,
    )

    install_neuronx_cc_hook()
    partition_name = nc.partition_id_tensor.name if nc.partition_id_tensor else None

    in_names, out_names, out_avals, zero_outs = [], [], [], []
    for alloc in nc.m.functions[0].allocations:
        if not isinstance(alloc, mybir.MemoryLocationSet):
            continue
        name = alloc.memorylocations[0].name
        if alloc.kind == "ExternalInput":
            if name != partition_name:
                in_names.append(name)
        elif alloc.kind == "ExternalOutput":
            out_names.append(name)
            shape = tuple(alloc.tensor_shape)
            dtype = mybir.dt.np(alloc.dtype)
            out_avals.append(jax.core.ShapedArray(shape, dtype))
            zero_outs.append(np.zeros(shape, dtype))
    n_params = len(in_names)
    n_outs = len(out_avals)
    all_names = in_names + out_names
    if partition_name is not None:
        all_names.append(partition_name)
    donate = tuple(range(n_params, n_params + n_outs))

    def _body(*args):
        operands = list(args)
        if partition_name is not None:
            operands.append(partition_id_tensor())
        outs = _bass_exec_p.bind(
            *operands,
            out_avals=tuple(out_avals),
            in_names=tuple(all_names),
            out_names=tuple(out_names),
            lowering_input_output_aliases=(),
            sim_require_finite=True,
            sim_require_nnan=True,
            nc=nc,
        )
        return tuple(outs)

    devices = jax.devices()[:n_cores]
    mesh = Mesh(np.asarray(devices), ("core",))
    in_specs = (PartitionSpec("core"),) * (n_params + n_outs)
    out_specs = (PartitionSpec("core"),) * len(out_names)
    fn = jax.jit(
        shard_map(_body, mesh=mesh, in_specs=in_specs, out_specs=out_specs,
                  check_rep=False),
        donate_argnums=donate,
        keep_unused=True,
    )
    sh = NamedSharding(mesh, PartitionSpec("core"))
    zeros_host = [
        np.zeros((n_cores * z.shape[0], *z.shape[1:]), z.dtype) for z in zero_outs
    ]

    def run(in_maps):
        concat_in = [
            np.concatenate([np.asarray(in_maps[c][nm]) for c in range(n_cores)], axis=0)
            for nm in in_names
        ]
        dev_in = [jax.device_put(a, sh) for a in concat_in]
        zeros = [jax.device_put(z, sh) for z in zeros_host]
        out = fn(*dev_in, *zeros)
        return [
            {
                nm: np.asarray(out[i]).reshape(n_cores, *out_avals[i].shape)[c]
                for i, nm in enumerate(out_names)
            }
            for c in range(n_cores)
        ]

    return run


# revision 22
# speedup vs baseline: 1.0667x; 1.0667x over previous
"""Trainium2 Bass kernel for knn_interpolate (k=3) + 2-layer MLP (FPModule).

Contract: kernel(**inputs) takes the FULL unsharded inputs (as produced by
setup_inputs()) and returns the FULL output, matching the reference:
    (h [16384,512] f32, pos_skip [16384,3] f32, batch_skip [16384] i64)

Sharding: 16384 queries split across 8 cores (2048 each). Batches are
contiguous & equal-sized (4 batches x 4096 queries / 1024 coarse points), so
core c serves queries [c*2048,(c+1)*2048) which all live in batch c//2; the
core only receives that batch's coarse points (masked KNN == per-batch KNN).

Per-core device algorithm:
  1. Dneg[q,c] = -||q-c||^2 via one K=5 matmul with augmented operands
     lhsT rows [qx,qy,qz,qq,1], rhs rows [2cx,2cy,2cz,-1,-cc].
  2. max8 over each [128,1024] row block -> top-8 candidates by PE distance;
     max_index -> their column indices (ties -> lowest index, like top_k).
  3. Take top-4 candidates, gather their [features | position] rows with ONE
     batched dma_gather, recompute exact fp32 d2 on-chip, re-rank exactly,
     compute 1/d2 weights from the exact values (masked to the exact top-3).
  4. xi = sum_j wn_j * xk_j (fused scalar_tensor_tensor chain), then
     h = relu(relu([xi,x_skip]@W1+b1)@W2+b2) with activations kept
     feature-major (PE transposes) so matmuls contract on partitions.
"""

import os
from contextlib import ExitStack

import numpy as np

NQ = 2048          # queries per core
NCC = 1024         # coarse points per core (one batch)
CF = 256           # coarse feature dim
SF = 128           # skip feature dim
HID = 512          # MLP hidden dim
KNN = 3
NCAND = 4          # candidates gathered per query (exact re-rank pool)
ELEM = 320         # padded row of the gather table: [x_c(256) | pos(3) | pad]
QT = NQ // 128     # 16 query tiles
CT = NCC // 128    # 8 coarse chunks
N_CORES = 8
NSLOT = QT * NCAND          # 64 gather slots per partition
NIDX = NSLOT * 128          # 8192 gathered rows per core

_CACHE = {}


def _build_nc():
    repeat = int(os.environ.get("KERNEL_REPEAT", "1"))

    import concourse.bass as bass
    import concourse.bacc as bacc
    import concourse.mybir as mybir
    import concourse.tile as tile

    f32 = mybir.dt.float32
    u32 = mybir.dt.uint32
    i16 = mybir.dt.int16
    AF = mybir.ActivationFunctionType
    OP = mybir.AluOpType
    X = mybir.AxisListType.X

    nc = bacc.Bacc("TRN2", target_bir_lowering=False, debug=False)

    posq = nc.dram_tensor("posq", [NQ, 3], f32, kind="ExternalInput").ap()
    xq = nc.dram_tensor("xq", [NQ, SF], f32, kind="ExternalInput").ap()
    posc = nc.dram_tensor("posc", [NCC, 3], f32, kind="ExternalInput").ap()
    xca = nc.dram_tensor("xca", [NCC, ELEM], f32, kind="ExternalInput").ap()
    w1 = nc.dram_tensor("w1", [CF + SF, HID], f32, kind="ExternalInput").ap()
    b1 = nc.dram_tensor("b1", [HID], f32, kind="ExternalInput").ap()
    w2 = nc.dram_tensor("w2", [HID, HID], f32, kind="ExternalInput").ap()
    b2 = nc.dram_tensor("b2", [HID], f32, kind="ExternalInput").ap()
    out = nc.dram_tensor("out", [NQ, HID], f32, kind="ExternalOutput").ap()
    stage = nc.dram_tensor("stage", [16 * 8 * NSLOT], i16).ap()  # idx bounce [c,g,s]

    from concourse.masks import make_identity

    with tile.TileContext(nc) as tc, ExitStack() as ctx:
        const = ctx.enter_context(tc.tile_pool(name="const", bufs=1))
        sb = ctx.enter_context(tc.tile_pool(name="sb", bufs=3))
        dsb_p = ctx.enter_context(tc.tile_pool(name="dsb", bufs=3))
        xi_p = ctx.enter_context(tc.tile_pool(name="xi", bufs=3))
        out_p = ctx.enter_context(tc.tile_pool(name="outp", bufs=3))
        psD = ctx.enter_context(tc.tile_pool(name="psD", bufs=2, space="PSUM"))
        psT = ctx.enter_context(tc.tile_pool(name="psT", bufs=2, space="PSUM"))
        psM = ctx.enter_context(tc.tile_pool(name="psM", bufs=2, space="PSUM"))

        # ---------- constants / persistent state ----------
        ident = const.tile([128, 128], f32)
        make_identity(nc, ident[:])

        w1_sb = const.tile([128, 3, HID], f32)
        nc.sync.dma_start(w1_sb[:], w1.rearrange("(kc p) h -> p kc h", p=128))
        w2_sb = const.tile([128, 4, HID], f32)
        nc.sync.dma_start(w2_sb[:], w2.rearrange("(kc p) h -> p kc h", p=128))
        b1_sb = const.tile([128, 4], f32)
        nc.sync.dma_start(b1_sb[:], b1.rearrange("(t p) -> p t", p=128))
        b2_sb = const.tile([1, HID], f32)
        nc.sync.dma_start(b2_sb[:], b2.rearrange("(o h) -> o h", o=1))
        ones1 = const.tile([1, 128], f32)
        nc.vector.memset(ones1[:], 1.0)

        posq_sb = const.tile([128, QT, 3], f32)
        nc.sync.dma_start(posq_sb[:], posq.rearrange("(t p) c -> p t c", p=128))
        posc_sb = const.tile([128, CT, 3], f32)
        nc.sync.dma_start(posc_sb[:], posc.rearrange("(t p) c -> p t c", p=128))

        # query positions replicated per candidate slot: [128, QT, NCAND, 3]
        posq_rep = const.tile([128, QT, NCAND, 3], f32)
        for j in range(NCAND):
            nc.vector.tensor_copy(posq_rep[:, :, j, :], posq_sb[:])

        # ---------- query-side augmented operand ----------
        # aug[p, t, :] = [qx, qy, qz, qq, 1, 0, 0, 0] for query t*128+p
        aug = const.tile([128, QT, 8], f32)
        nc.vector.tensor_copy(aug[:, :, 0:3], posq_sb[:])
        sqq = sb.tile([128, QT, 3], f32, tag="sqq")
        nc.vector.tensor_mul(sqq[:], posq_sb[:], posq_sb[:])
        nc.vector.reduce_sum(aug[:, :, 3:4], sqq[:], axis=X)
        nc.vector.memset(aug[:, :, 4:5], 1.0)
        nc.vector.memset(aug[:, :, 5:8], 0.0)

        augT = const.tile([8, QT * 128], f32)
        for t in range(QT):
            tp = psT.tile([8, 128], f32, tag="tps")
            nc.tensor.transpose(tp[:], aug[:, t, :], ident[:])
            nc.scalar.activation(augT[:, t * 128:(t + 1) * 128], tp[:], AF.Copy)

        # ---------- coarse-side rhs operand [8, NCC] ----------
        # rows: [2cx, 2cy, 2cz, -1, -cc, 0, 0, 0]
        cst = const.tile([128, CT, 8], f32)
        nc.scalar.mul(cst[:, :, 0:3], posc_sb[:], 2.0)
        nc.vector.memset(cst[:, :, 3:4], -1.0)
        sqc = sb.tile([128, CT, 3], f32, tag="sqc")
        nc.vector.tensor_mul(sqc[:], posc_sb[:], posc_sb[:])
        nc.vector.reduce_sum(cst[:, :, 4:5], sqc[:], axis=X, negate=True)
        nc.vector.memset(cst[:, :, 5:8], 0.0)
        rhsD = const.tile([8, NCC], f32)
        for u in range(CT):
            tp = psT.tile([8, 128], f32, tag="tps")
            nc.tensor.transpose(tp[:], cst[:, u, :], ident[:])
            nc.scalar.activation(rhsD[:, u * 128:(u + 1) * 128], tp[:], AF.Copy)

        for _rep in range(repeat):
            # ---------- distances + top-8 candidates ----------
            mx = const.tile([128, QT * 8], f32)
            ix = const.tile([128, QT * 8], u32)
            for t in range(QT):
                psd = psD.tile([128, NCC], f32, tag="psd")
                lhs = augT[:, t * 128:(t + 1) * 128]
                nc.tensor.matmul(psd[:, 0:512], lhs, rhsD[:, 0:512], start=True, stop=True)
                nc.tensor.matmul(psd[:, 512:1024], lhs, rhsD[:, 512:1024], start=True, stop=True)
                dsb = dsb_p.tile([128, NCC], f32, tag="dsb")
                nc.scalar.activation(dsb[:], psd[:], AF.Copy)
                nc.vector.max(mx[:, t * 8:(t + 1) * 8], dsb[:])
                nc.vector.max_index(ix[:, t * 8:(t + 1) * 8], mx[:, t * 8:(t + 1) * 8], dsb[:])

            # ---------- build wrapped int16 index table for dma_gather ----------
            # slot layout: slot = t*NCAND + j holds candidate j of tile t.
            ixc = const.tile([128, NSLOT], i16)
            ix_v = ix[:].rearrange("p (t e) -> p t e", e=8)
            nc.vector.tensor_copy(ixc[:].rearrange("p (t j) -> p t j", j=NCAND),
                                  ix_v[:, :, 0:NCAND])
            # hop1: straight row-major store; stage[p * NSLOT + s] = ixc[p, s]
            nc.sync.dma_start(
                stage.rearrange("(p s) -> p s", s=NSLOT), ixc[:]
            )
            # hop2: per 16-partition group, load the cross-partition regroup
            # raw[c, g*NSLOT + s] = stage[(16g + c) * NSLOT + s]
            raw = const.tile([128, 8 * NSLOT], i16)
            for grp in range(8):
                src_ap = bass.AP(
                    tensor=stage.tensor,
                    offset=0,
                    ap=[[NSLOT, 16], [16 * NSLOT, 8], [1, NSLOT]],
                )
                nc.sync.dma_start(raw[16 * grp:16 * (grp + 1), :], src_ap)
            # hop3: in-SBUF permute (g, s) -> wrapped order (s, g) on DVE
            table = const.tile([128, NIDX // 16], i16)
            nc.vector.tensor_copy(
                table[:].rearrange("p (s g) -> p g s", g=8),
                raw[:].rearrange("p (g s) -> p g s", g=8),
            )

            # ---------- batched gather of [features | position] ----------
            # split into 1024-row chunks (large single gathers wedge the
            # SWDGE descriptor ring)
            xk = const.tile([128, NSLOT, ELEM], f32)
            for kk in range(NIDX // 1024):
                nc.gpsimd.dma_gather(
                    out_ap=xk[:, 8 * kk:8 * (kk + 1), :],
                    in_ap=xca[:, :],
                    idxs_ap=table[:, 64 * kk:64 * (kk + 1)],
                    num_idxs=1024,
                    num_idxs_reg=1024,
                    elem_size=ELEM,
                )

            # ---------- exact re-rank + interpolation weights ----------
            diff = const.tile([128, NSLOT, 3], f32)
            nc.vector.tensor_sub(
                diff[:], xk[:, :, CF:CF + 3],
                posq_rep[:].rearrange("p t j c -> p (t j) c"),
            )
            sqd = const.tile([128, NSLOT, 3], f32)
            nc.vector.tensor_mul(sqd[:], diff[:], diff[:])
            d2e = const.tile([128, QT, NCAND], f32)
            nc.vector.reduce_sum(d2e[:], sqd[:], axis=X)

            # negated + padded to 8 for per-tile max8 exact sort
            dn = const.tile([128, QT, 8], f32)
            nc.vector.memset(dn[:, :, NCAND:8], -1e30)
            nc.vector.tensor_scalar(dn[:, :, 0:NCAND], d2e[:], -1.0, None, op0=OP.mult)
            mxe = const.tile([128, QT * 8], f32)
            for t in range(QT):
                nc.vector.max(mxe[:, t * 8:(t + 1) * 8], dn[:, t, :])
            mxe_v = mxe[:].rearrange("p (t e) -> p t e", e=8)

            # exact third-smallest per query (selection threshold)
            v3e = const.tile([128, QT], f32)
            nc.vector.tensor_scalar(v3e[:], mxe_v[:, :, 2], -1.0, None, op0=OP.mult)
            # masked inverse-distance weights over the NCAND candidates
            d2c = const.tile([128, QT, NCAND], f32)
            nc.vector.tensor_scalar(d2c[:], d2e[:], 1e-16, None, op0=OP.max)
            wraw = const.tile([128, QT, NCAND], f32)
            nc.vector.reciprocal(wraw[:], d2c[:])
            mask = const.tile([128, QT, NCAND], f32)
            for j in range(NCAND):
                nc.vector.tensor_tensor(
                    mask[:, :, j], d2e[:, :, j], v3e[:], op=OP.is_le
                )
            wk = const.tile([128, QT, NCAND], f32)
            nc.vector.tensor_mul(wk[:], wraw[:], mask[:])
            wsum = const.tile([128, QT], f32)
            nc.vector.reduce_sum(wsum[:], wk[:], axis=X)
            wni = const.tile([128, QT], f32)
            nc.vector.reciprocal(wni[:], wsum[:])
            wn = const.tile([128, QT, NCAND], f32)
            for j in range(NCAND):
                nc.vector.tensor_mul(wn[:, :, j], wk[:, :, j], wni[:])

            # ---------- weighted sum + transpose to feature-major ----------
            xiT = const.tile([128, 2, NQ], f32)
            for t in range(QT):
                s0 = t * NCAND
                xi = xi_p.tile([128, CF], f32, tag="xi")
                nc.vector.tensor_scalar_mul(xi[:], xk[:, s0, 0:CF], wn[:, t, 0:1])
                for j in range(1, NCAND):
                    nc.vector.scalar_tensor_tensor(
                        xi[:], xk[:, s0 + j, 0:CF], wn[:, t, j:j + 1], xi[:],
                        op0=OP.mult, op1=OP.add,
                    )
                for fc in range(2):
                    tp = psT.tile([128, 128], f32, tag="tps")
                    nc.tensor.transpose(tp[:], xi[:, fc * 128:(fc + 1) * 128], ident[:])
                    nc.scalar.activation(xiT[:, fc, t * 128:(t + 1) * 128], tp[:], AF.Copy)

            # x_skip transposed to feature-major
            xsT = const.tile([128, NQ], f32)
            for t in range(QT):
                xst = sb.tile([128, SF], f32, tag="xst")
                nc.sync.dma_start(xst[:], xq[t * 128:(t + 1) * 128, :])
                tp = psT.tile([128, 128], f32, tag="tps")
                nc.tensor.transpose(tp[:], xst[:], ident[:])
                nc.scalar.activation(xsT[:, t * 128:(t + 1) * 128], tp[:], AF.Copy)

            # ---------- MLP layer 1: h1T[p, ht, q] ----------
            h1T = const.tile([128, 4, NQ], f32)
            for qc in range(NQ // 512):
                qs = slice(qc * 512, (qc + 1) * 512)
                for ht in range(4):
                    ps = psM.tile([128, 512], f32, tag="psm")
                    for kc in range(3):
                        rhs = xiT[:, kc, qs] if kc < 2 else xsT[:, qs]
                        nc.tensor.matmul(
                            ps[:],
                            w1_sb[:, kc, ht * 128:(ht + 1) * 128],
                            rhs,
                            start=(kc == 0),
                            stop=(kc == 2),
                        )
                    nc.scalar.activation(
                        h1T[:, ht, qs], ps[:], AF.Relu, bias=b1_sb[:, ht:ht + 1]
                    )

            # ---------- MLP layer 2 (query-major out) ----------
            for t in range(QT):
                ps2 = psM.tile([128, 512], f32, tag="psm")
                for kc in range(4):
                    nc.tensor.matmul(
                        ps2[:],
                        h1T[:, kc, t * 128:(t + 1) * 128],
                        w2_sb[:, kc, :],
                        start=(kc == 0),
                        stop=False,
                    )
                nc.tensor.matmul(ps2[:], ones1[:], b2_sb[:], start=False, stop=True)
                ot = out_p.tile([128, HID], f32, tag="ot")
                nc.scalar.activation(ot[:], ps2[:], AF.Relu)
                nc.sync.dma_start(out[t * 128:(t + 1) * 128, :], ot[:])

    nc.finalize()
    return nc


def get_nc():
    if "nc" not in _CACHE:
        _CACHE["nc"] = _build_nc()
    return _CACHE["nc"]


def make_in_maps(x, pos, x_skip, pos_skip, W1, b1, W2, b2):
    c32 = lambda a: np.ascontiguousarray(np.asarray(a), dtype=np.float32)
    x = c32(x)
    pos = c32(pos)
    xca_full = np.zeros((x.shape[0], ELEM), np.float32)
    xca_full[:, :CF] = x
    xca_full[:, CF:CF + 3] = pos
    maps = []
    for c in range(N_CORES):
        b = (c * NQ) // (len(pos_skip) // 4)  # batch of this core's query slice
        maps.append(
            {
                "posq": c32(pos_skip[c * NQ:(c + 1) * NQ]),
                "xq": c32(x_skip[c * NQ:(c + 1) * NQ]),
                "posc": c32(pos[b * NCC:(b + 1) * NCC]),
                "xca": np.ascontiguousarray(xca_full[b * NCC:(b + 1) * NCC]),
                "w1": c32(W1),
                "b1": c32(b1),
                "w2": c32(W2),
                "b2": c32(b2),
            }
        )
    return maps


def _expected_layout(batch, batch_skip, k):
    """True iff inputs match the layout this kernel was compiled for."""
    batch = np.asarray(batch)
    batch_skip = np.asarray(batch_skip)
    if int(k) != KNN:
        return False
    if batch.shape != (4096,) or batch_skip.shape != (16384,):
        return False
    exp_b = np.repeat(np.arange(4), 1024)
    exp_bs = np.repeat(np.arange(4), 4096)
    return bool(np.array_equal(batch, exp_b) and np.array_equal(batch_skip, exp_bs))


def _numpy_fallback(x, pos, batch, x_skip, pos_skip, batch_skip, W1, b1, W2, b2, k):
    x, pos, x_skip, pos_skip = (np.asarray(a, np.float64) for a in (x, pos, x_skip, pos_skip))
    batch = np.asarray(batch)
    batch_skip = np.asarray(batch_skip)
    d2 = (
        (pos_skip**2).sum(-1)[:, None]
        + (pos**2).sum(-1)[None, :]
        - 2.0 * (pos_skip @ pos.T)
    )
    d2 = np.maximum(d2, 0.0)
    d2 = d2 + np.where(batch_skip[:, None] != batch[None, :], 1e10, 0.0)
    idx = np.argsort(d2, axis=1, kind="stable")[:, : int(k)]
    d2k = np.take_along_axis(d2, idx, axis=1)
    w = 1.0 / np.clip(d2k, 1e-16, None)
    xk = x[idx]
    xi = (w[..., None] * xk).sum(1) / w.sum(1, keepdims=True)
    h = np.concatenate([xi, np.asarray(x_skip, np.float64)], axis=1)
    h = np.maximum(h @ np.asarray(W1, np.float64) + np.asarray(b1, np.float64), 0.0)
    h = np.maximum(h @ np.asarray(W2, np.float64) + np.asarray(b2, np.float64), 0.0)
    return (
        h.astype(np.float32),
        np.asarray(pos_skip, np.float32),
        batch_skip,
    )


def kernel(x, pos, batch, x_skip, pos_skip, batch_skip, W1, b1, W2, b2, k, **run_kw):
    x = np.asarray(x)
    pos = np.asarray(pos)
    x_skip = np.asarray(x_skip)
    pos_skip = np.asarray(pos_skip)
    if not _expected_layout(batch, batch_skip, k):
        return _numpy_fallback(
            x, pos, batch, x_skip, pos_skip, batch_skip, W1, b1, W2, b2, k
        )

    in_maps = make_in_maps(x, pos, x_skip, pos_skip, W1, b1, W2, b2)
    try:
        if "runner" not in _CACHE:
            _CACHE["runner"] = make_runner(get_nc(), n_cores=N_CORES)
        results = _CACHE["runner"](in_maps)
    except Exception:
        _CACHE.pop("runner", None)
        from concourse.bass_utils import run_bass_kernel_spmd

        results = run_bass_kernel_spmd(get_nc(), in_maps, list(range(N_CORES))).results
    h = np.concatenate([results[c]["out"] for c in range(N_CORES)], axis=0)
    return h, np.asarray(pos_skip), np.asarray(batch_skip)


def make_runner(nc, n_cores=8):
    """Build a reusable jitted SPMD executor for `nc` on the first n_cores
    devices. Returns run(in_maps) -> list[dict[name, np.ndarray]]."""
    import jax
    from jax.sharding import Mesh, PartitionSpec, NamedSharding

    try:
        from jax.experimental.shard_map import shard_map
    except ImportError:
        from jax import shard_map
    import concourse.mybir as mybir
    from concourse.bass2jax import (
        _bass_exec_p,
        install_neuronx_cc_hook,
        partition_id_tensor,
    )

    install_neuronx_cc_hook()
    partition_name = nc.partition_id_tensor.name if nc.partition_id_tensor else None

    in_names, out_names, out_avals, zero_outs = [], [], [], []
    for alloc in nc.m.functions[0].allocations:
        if not isinstance(alloc, mybir.MemoryLocationSet):
            continue
        name = alloc.memorylocations[0].name
        if alloc.kind == "ExternalInput":
            if name != partition_name:
                in_names.append(name)
        elif alloc.kind == "ExternalOutput":
            out_names.append(name)
            shape = tuple(alloc.tensor_shape)
            dtype = mybir.dt.np(alloc.dtype)
            out_avals.append(jax.core.ShapedArray(shape, dtype))
            zero_outs.append(np.zeros(shape, dtype))
    n_params = len(in_names)
    n_outs = len(out_avals)
    all_names = in_names + out_names
    if partition_name is not None:
        all_names.append(partition_name)
    donate = tuple(range(n_params, n_params + n_outs))

    def _body(*args):
        operands = list(args)
        if partition_name is not None:
            operands.append(partition_id_tensor())
        outs = _bass_exec_p.bind(
            *operands,
            out_avals=tuple(out_avals),
            in_names=tuple(all_names),
            out_names=tuple(out_names),
            lowering_input_output_aliases=(),
            sim_require_finite=True,
            sim_require_nnan=True,
            nc=nc,
        )
        return tuple(outs)

    devices = jax.devices()[:n_cores]
    mesh = Mesh(np.asarray(devices), ("core",))
    in_specs = (PartitionSpec("core"),) * (n_params + n_outs)
    out_specs = (PartitionSpec("core"),) * len(out_names)
    fn = jax.jit(
        shard_map(_body, mesh=mesh, in_specs=in_specs, out_specs=out_specs,
                  check_rep=False),
        donate_argnums=donate,
        keep_unused=True,
    )
    sh = NamedSharding(mesh, PartitionSpec("core"))
    zeros_host = [
        np.zeros((n_cores * z.shape[0], *z.shape[1:]), z.dtype) for z in zero_outs
    ]

    def run(in_maps):
        concat_in = [
            np.concatenate([np.asarray(in_maps[c][nm]) for c in range(n_cores)], axis=0)
            for nm in in_names
        ]
        dev_in = [jax.device_put(a, sh) for a in concat_in]
        zeros = [jax.device_put(z, sh) for z in zeros_host]
        out = fn(*dev_in, *zeros)
        return [
            {
                nm: np.asarray(out[i]).reshape(n_cores, *out_avals[i].shape)[c]
                for i, nm in enumerate(out_names)
            }
            for c in range(n_cores)
        ]

    return run
